# revision 7
# baseline (speedup 1.0000x reference)
"""BiLSTM-CRF NLL kernel for Trainium2, 8 NeuronCores (SPMD).

Strategy:
  - Phase A (all 8 cores, token-sharded): embedding gather + input projections
    for both LSTM directions -> xp chunks, AllGather so the two scan cores see
    the full sequence.  The backward direction is handled by feeding the scan
    time-reversed xp (host prepares reversed token indices), so every core runs
    the *same* forward-scan program.
  - Recurrence: even cores scan forward xp, odd cores scan reversed xp
    (per-core input data selects which).  Whh stays SBUF-resident in bf16;
    per step 64 accumulating matmuls (16 M-tiles x 4 K-tiles, N=32).
  - Emissions: core 0 projects its h-history with the forward half of Wout,
    core 1 with the backward half (other cores get zero weights);
    AllReduce(add) assembles full emissions on every core.  The backward
    core's time-reversal is handled by storing h both in scan order and
    reversed order and weighting the right copy.
  - CRF: denominator via exp-domain scan q_{t+1} = (expT^T q_t) * exp(em-c)
    (one tiny matmul + one vector mul per step, stationary weights); numerator
    via host-precomputed tag gathers + on-device em*onehot reduction.
"""

import numpy as np
import ml_dtypes
from contextlib import ExitStack

import concourse.bass as bass
from concourse import bacc, mybir
from concourse.bass_utils import run_bass_kernel_spmd
from concourse.tile import TileContext

F32 = mybir.dt.float32
BF16 = mybir.dt.bfloat16
I32 = mybir.dt.int32
AF = mybir.ActivationFunctionType
OP = mybir.AluOpType

V, T, E, H, B = 50000, 45, 256, 512, 32
P = 128
NCORE = 8
G4 = 4 * H            # 2048 gate rows per direction
M16 = G4 // P         # 16 M-tiles
KC = H // P           # 4 K-tiles over hidden
KT = E // P           # 2 K-tiles over embedding
CLOG = 3.85           # per-step log shift for the CRF exp-domain scan

STAGES = ["pa", "ag", "rec", "emc", "em", "full"]


def build_nc(S, stage="full"):
    lvl = STAGES.index(stage)
    SC = S // NCORE       # scan steps per chunk
    TOK = SC * B          # tokens per chunk
    NG = max(1, TOK // P) # 128-token gather groups per chunk
    NPC = min(512, TOK)   # projection N chunk
    NCKP = TOK // NPC
    NBLK = 8 if S % 8 == 0 and S >= 8 else 1   # emission time blocks
    SBK = S // NBLK
    CB = SBK * B          # emission cols per block
    NPCE = min(512, CB)
    NCKE = CB // NPCE

    nc = bacc.Bacc(None)

    # ---------------- I/O ----------------
    emb = nc.dram_tensor("emb", [V, E], F32, kind="ExternalInput")
    id128 = nc.dram_tensor("id128", [P, P], BF16, kind="ExternalInput")
    xf_idx = nc.dram_tensor("xf_idx", [NG, P, 1], I32, kind="ExternalInput")
    xb_idx = nc.dram_tensor("xb_idx", [NG, P, 1], I32, kind="ExternalInput")
    wihT_f = nc.dram_tensor("wihT_f", [P, KT * G4], BF16, kind="ExternalInput")
    wihT_b = nc.dram_tensor("wihT_b", [P, KT * G4], BF16, kind="ExternalInput")
    pbias_f = nc.dram_tensor("pbias_f", [P, M16], F32, kind="ExternalInput")
    pbias_b = nc.dram_tensor("pbias_b", [P, M16], F32, kind="ExternalInput")
    whhT = nc.dram_tensor("whhT", [P, KC * G4], BF16, kind="ExternalInput")
    xprows0 = nc.dram_tensor("xprows0", [M16, P, 1], I32, kind="ExternalInput")
    xprowsg = nc.dram_tensor("xprowsg", [NCORE - 1, M16, P, 1], I32, kind="ExternalInput")
    woutT_A = nc.dram_tensor("woutT_A", [P, KC * T], BF16, kind="ExternalInput")
    woutT_B = nc.dram_tensor("woutT_B", [P, KC * T], BF16, kind="ExternalInput")
    bout_h = nc.dram_tensor("bout_h", [T, 1], F32, kind="ExternalInput")
    expT = nc.dram_tensor("expT", [T, T], BF16, kind="ExternalInput")
    expS = nc.dram_tensor("expS", [T, 1], F32, kind="ExternalInput")
    expE = nc.dram_tensor("expE", [T, 1], F32, kind="ExternalInput")
    oh = nc.dram_tensor("oh", [T, S * B], F32, kind="ExternalInput")
    numconst = nc.dram_tensor("numconst", [1, B], F32, kind="ExternalInput")
    out = nc.dram_tensor("out", [1, 1], F32, kind="ExternalOutput")

    # ---------------- internal DRAM ----------------
    xp_contrib = nc.dram_tensor(
        "xp_contrib", [2, M16, P, TOK], BF16,
        kind=("ExternalOutput" if stage == "pa" else "Internal"))
    xp_all = nc.dram_tensor("xp_all", [NCORE, 2, M16, P, TOK], BF16, addr_space="Shared")
    hs_kind = "ExternalOutput" if stage == "rec" else "Internal"
    hs_fwd = nc.dram_tensor("hs_fwd", [KC, P, S, B], BF16, kind=hs_kind)
    hs_rev = nc.dram_tensor("hs_rev", [KC, P, S, B], BF16, kind=hs_kind)
    em_contrib = nc.dram_tensor(
        "em_contrib", [T, S * B], F32,
        kind=("ExternalOutput" if stage == "emc" else "Internal"))
    em_all = nc.dram_tensor("em_all", [T, S * B], F32, addr_space="Shared")

    dims = dict(S=S, SC=SC, TOK=TOK, NG=NG, NPC=NPC, NCKP=NCKP,
                NBLK=NBLK, SBK=SBK, CB=CB, NPCE=NPCE, NCKE=NCKE)
    tens = dict(emb=emb, id128=id128, xf_idx=xf_idx, xb_idx=xb_idx,
                wihT_f=wihT_f, wihT_b=wihT_b, pbias_f=pbias_f, pbias_b=pbias_b,
                whhT=whhT, xprows0=xprows0, xprowsg=xprowsg,
                woutT_A=woutT_A, woutT_B=woutT_B, bout_h=bout_h,
                expT=expT, expS=expS, expE=expE, oh=oh, numconst=numconst,
                out=out, xp_contrib=xp_contrib, xp_all=xp_all,
                hs_fwd=hs_fwd, hs_rev=hs_rev,
                em_contrib=em_contrib, em_all=em_all)
    v = {**dims, **tens}

    with ExitStack() as top:
        tc = top.enter_context(TileContext(nc))
        wp = top.enter_context(tc.tile_pool(name="weights", bufs=1))

        whh_sb = wp.tile([P, KC * G4], BF16)
        nc.sync.dma_start(whh_sb[:], whhT[:])

        _phase_a(nc, tc, wp, v)
        if lvl >= 1:
            nc.gpsimd.collective_compute(
                "AllGather", OP.bypass,
                ins=[xp_contrib[:]], outs=[xp_all[:]],
                replica_groups=[list(range(NCORE))],
            )
        if lvl == 1:
            dbg = nc.dram_tensor("dbg_ag", [NCORE, 2, M16, P, TOK], BF16,
                                 kind="ExternalOutput")
            with tc.tile_pool(name="agdbg", bufs=2) as adp:
                for r in range(NCORE):
                    bt = adp.tile([P, 2 * M16 * TOK], BF16, tag="agb")
                    nc.sync.dma_start(bt[:], xp_all[r].transpose([2, 0, 1, 3]))
                    nc.sync.dma_start(dbg[r].transpose([2, 0, 1, 3]), bt[:])
        if lvl >= 2:
            _recurrence(nc, tc, wp, whh_sb, v)
        if lvl >= 3:
            _emissions(nc, tc, wp, v)
        if lvl >= 4:
            nc.gpsimd.collective_compute(
                "AllReduce", OP.add,
                ins=[em_contrib[:]], outs=[em_all[:]],
                replica_groups=[list(range(NCORE))],
            )
        if lvl == 4:
            dbg = nc.dram_tensor("dbg_em", [T, S * B], F32, kind="ExternalOutput")
            with tc.tile_pool(name="emdbg", bufs=1) as edp:
                bt = edp.tile([T, S * B], F32)
                nc.sync.dma_start(bt[:], em_all[:])
                nc.sync.dma_start(dbg[:], bt[:])
        if lvl >= 5:
            _crf(nc, tc, wp, v)
        else:
            with tc.tile_pool(name="zout", bufs=1) as zp:
                zt = zp.tile([1, 1], F32)
                nc.vector.memset(zt[:], 0.0)
                nc.sync.dma_start(out[:], zt[:])

    nc.compile()
    return nc


def _phase_a(nc, tc, wp, v):
    TOK, NG, NPC, NCKP = v["TOK"], v["NG"], v["NPC"], v["NCKP"]
    emb, id128 = v["emb"], v["id128"]
    xp_contrib = v["xp_contrib"]

    with ExitStack() as pa:
        ep = pa.enter_context(tc.tile_pool(name="embT", bufs=2))
        gp = pa.enter_context(tc.tile_pool(name="gath", bufs=3))
        xo = pa.enter_context(tc.tile_pool(name="xpout", bufs=4))
        pp = pa.enter_context(tc.tile_pool(name="pa_ps", bufs=2, space="PSUM"))
        tp = pa.enter_context(tc.tile_pool(name="pa_tps", bufs=2, space="PSUM"))

        id_sb = wp.tile([P, P], BF16)
        nc.sync.dma_start(id_sb[:], id128[:])
        wihf_sb = wp.tile([P, KT * G4], BF16)
        nc.sync.dma_start(wihf_sb[:], v["wihT_f"][:])
        wihb_sb = wp.tile([P, KT * G4], BF16)
        nc.sync.dma_start(wihb_sb[:], v["wihT_b"][:])
        biasf_sb = wp.tile([P, M16], F32)
        nc.sync.dma_start(biasf_sb[:], v["pbias_f"][:])
        biasb_sb = wp.tile([P, M16], F32)
        nc.sync.dma_start(biasb_sb[:], v["pbias_b"][:])

        for d, (xidx, wih_sb, bias_sb) in enumerate(
            [(v["xf_idx"], wihf_sb, biasf_sb), (v["xb_idx"], wihb_sb, biasb_sb)]
        ):
            embT_sb = ep.tile([P, KT * TOK], BF16, tag="embT")
            for g in range(NG):
                idx_sb = gp.tile([P, 1], I32, tag="gidx")
                nc.sync.dma_start(idx_sb[:], xidx[g])
                gath = gp.tile([P, E], F32, tag="gath")
                nc.gpsimd.indirect_dma_start(
                    out=gath[:], out_offset=None, in_=emb[:],
                    in_offset=bass.IndirectOffsetOnAxis(ap=idx_sb[:, :1], axis=0),
                )
                gbf = gp.tile([P, E], BF16, tag="gbf")
                nc.vector.tensor_copy(gbf[:], gath[:])
                for kt in range(KT):
                    tps = tp.tile([P, P], BF16, tag="tps")
                    nc.tensor.transpose(tps[:], gbf[:, kt * P:(kt + 1) * P], id_sb[:])
                    nc.vector.tensor_copy(
                        embT_sb[:, kt * TOK + g * P: kt * TOK + (g + 1) * P], tps[:]
                    )
            for m in range(M16):
                for nk in range(NCKP):
                    psA = pp.tile([P, NPC], F32, tag="psproj")
                    for kt in range(KT):
                        nc.tensor.matmul(
                            psA[:],
                            lhsT=wih_sb[:, kt * G4 + m * P: kt * G4 + (m + 1) * P],
                            rhs=embT_sb[:, kt * TOK + nk * NPC: kt * TOK + (nk + 1) * NPC],
                            start=(kt == 0), stop=(kt == KT - 1),
                        )
                    xps = xo.tile([P, NPC], BF16, tag="xpout")
                    if m % 2 == 0:
                        nc.scalar.activation(
                            xps[:], psA[:], AF.Identity, bias=bias_sb[:, m:m + 1]
                        )
                    else:
                        nc.vector.tensor_scalar_add(
                            xps[:], in0=psA[:], scalar1=bias_sb[:, m:m + 1]
                        )
                    nc.sync.dma_start(
                        xp_contrib[d, m, :, nk * NPC:(nk + 1) * NPC], xps[:]
                    )


def _recurrence(nc, tc, wp, whh_sb, v):
    S, SC, TOK = v["S"], v["SC"], v["TOK"]
    xp_contrib, xp_all = v["xp_contrib"], v["xp_all"]
    xprows0, xprowsg = v["xprows0"], v["xprowsg"]
    hs_fwd, hs_rev = v["hs_fwd"], v["hs_rev"]

    xp_c_rows = xp_contrib.rearrange("d m p t -> (d m p) t")
    xp_a_rows = xp_all.rearrange("r d m p t -> (r d m p) t")
    hs_fwd_t = hs_fwd.transpose([1, 0, 2, 3])  # [P, KC, S, B] view
    hs_rev_t = hs_rev.transpose([1, 0, 2, 3])

    with ExitStack() as pr:
        xch = pr.enter_context(tc.tile_pool(name="xpch", bufs=2))
        iop = pr.enter_context(tc.tile_pool(name="xpo", bufs=2))
        st = pr.enter_context(tc.tile_pool(name="state", bufs=2))
        sg = pr.enter_context(tc.tile_pool(name="sgates", bufs=2))
        pg = pr.enter_context(tc.tile_pool(name="rec_ps", bufs=2, space="PSUM"))

        def load_chunk(u):
            xpo = iop.tile([P, M16], I32, tag="xpo")
            src = xprows0 if u == 0 else xprowsg[u - 1]
            nc.sync.dma_start(xpo[:], src.transpose([1, 0, 2]))
            xt = xch.tile([P, M16 * TOK], BF16, tag="xpch")
            rows = xp_c_rows if u == 0 else xp_a_rows
            for m in range(M16):
                nc.gpsimd.indirect_dma_start(
                    out=xt[:, m * TOK:(m + 1) * TOK], out_offset=None, in_=rows,
                    in_offset=bass.IndirectOffsetOnAxis(ap=xpo[:, m:m + 1], axis=0),
                )
            return xt

        chunks = {0: load_chunk(0)}
        h_prev = st.tile([P, P], BF16, tag="h")
        nc.vector.memset(h_prev[:], 0.0)
        c_prev = st.tile([P, P], F32, tag="c")
        nc.vector.memset(c_prev[:], 0.0)

        for t in range(S):
            u, tl = divmod(t, SC)
            if tl == 0 and u + 1 < NCORE:
                chunks[u + 1] = load_chunk(u + 1)
            xt = chunks[u]
            ps = pg.tile([P, 4 * P], F32, tag="gpsum")
            for m in range(M16):
                lo = m * P
                for kc in range(KC):
                    nc.tensor.matmul(
                        ps[:, 32 * m: 32 * m + 32],
                        lhsT=whh_sb[:, kc * G4 + lo: kc * G4 + lo + P],
                        rhs=h_prev[:, 32 * kc: 32 * kc + 32],
                        start=(kc == 0), stop=(kc == KC - 1),
                    )
            gates = sg.tile([P, 4 * P], F32, tag="gates")
            xt_v = xt.rearrange("p (m tok) -> p m tok", m=M16)[:, :, tl * B:(tl + 1) * B]
            nc.vector.tensor_add(
                gates.rearrange("p (m b) -> p m b", m=M16),
                ps.rearrange("p (m b) -> p m b", m=M16),
                xt_v,
            )
            act = sg.tile([P, 4 * P], F32, tag="act")
            nc.scalar.activation(act[:, 0:384], gates[:, 0:384], AF.Sigmoid)
            nc.scalar.activation(act[:, 384:512], gates[:, 384:512], AF.Tanh)
            tmp = st.tile([P, P], F32, tag="tmp")
            nc.vector.tensor_mul(tmp[:], act[:, 0:P], act[:, 384:512])
            tmp2 = st.tile([P, P], F32, tag="tmp2")
            nc.vector.tensor_mul(tmp2[:], act[:, P:2 * P], c_prev[:])
            c_new = st.tile([P, P], F32, tag="c")
            nc.vector.tensor_add(c_new[:], tmp[:], tmp2[:])
            tanc = st.tile([P, P], F32, tag="tanc")
            nc.scalar.activation(tanc[:], c_new[:], AF.Tanh)
            h_new = st.tile([P, P], BF16, tag="h")
            nc.vector.tensor_mul(h_new[:], act[:, 2 * P:3 * P], tanc[:])
            hv = h_new.rearrange("p (kc b) -> p kc b", kc=KC)
            nc.sync.dma_start(hs_fwd_t[:, :, t, :], hv)
            nc.sync.dma_start(hs_rev_t[:, :, S - 1 - t, :], hv)
            h_prev, c_prev = h_new, c_new


def _emissions(nc, tc, wp, v):
    S, NBLK, SBK, CB, NPCE, NCKE = (v["S"], v["NBLK"], v["SBK"], v["CB"],
                                    v["NPCE"], v["NCKE"])
    hs_fwd, hs_rev, em_contrib = v["hs_fwd"], v["hs_rev"], v["em_contrib"]

    with ExitStack() as pe:
        hp2 = pe.enter_context(tc.tile_pool(name="hsld", bufs=2))
        emp = pe.enter_context(tc.tile_pool(name="emsb", bufs=1))
        pep = pe.enter_context(tc.tile_pool(name="em_ps", bufs=2, space="PSUM"))

        woutA_sb = wp.tile([P, KC * T], BF16)
        nc.sync.dma_start(woutA_sb[:], v["woutT_A"][:])
        woutB_sb = wp.tile([P, KC * T], BF16)
        nc.sync.dma_start(woutB_sb[:], v["woutT_B"][:])
        bout_sb = wp.tile([T, 1], F32)
        nc.sync.dma_start(bout_sb[:], v["bout_h"][:])

        em_sb = emp.tile([T, S * B], F32)
        for q in range(NBLK):
            hsA, hsB = [], []
            for kc in range(KC):
                ta = hp2.tile([P, CB], BF16, tag=f"hsA{kc}")
                nc.sync.dma_start(ta[:], hs_fwd[kc, :, q * SBK:(q + 1) * SBK, :])
                hsA.append(ta)
                tb = hp2.tile([P, CB], BF16, tag=f"hsB{kc}")
                nc.sync.dma_start(tb[:], hs_rev[kc, :, q * SBK:(q + 1) * SBK, :])
                hsB.append(tb)
            for nk in range(NCKE):
                pse = pep.tile([T, NPCE], F32, tag="emps")
                for kc in range(KC):
                    nc.tensor.matmul(
                        pse[:], lhsT=woutA_sb[:, kc * T:(kc + 1) * T],
                        rhs=hsA[kc][:, nk * NPCE:(nk + 1) * NPCE],
                        start=(kc == 0), stop=False,
                    )
                for kc in range(KC):
                    nc.tensor.matmul(
                        pse[:], lhsT=woutB_sb[:, kc * T:(kc + 1) * T],
                        rhs=hsB[kc][:, nk * NPCE:(nk + 1) * NPCE],
                        start=False, stop=(kc == KC - 1),
                    )
                nc.scalar.activation(
                    em_sb[:, q * CB + nk * NPCE: q * CB + (nk + 1) * NPCE],
                    pse[:], AF.Identity, bias=bout_sb[:, 0:1],
                )
        nc.sync.dma_start(em_contrib[:], em_sb[:])


def _crf(nc, tc, wp, v):
    S = v["S"]
    em_all, oh, numconst = v["em_all"], v["oh"], v["numconst"]
    expT, expS, expE, out = v["expT"], v["expS"], v["expE"], v["out"]

    with ExitStack() as pc:
        crf = pc.enter_context(tc.tile_pool(name="crf", bufs=1))
        qp = pc.enter_context(tc.tile_pool(name="qtile", bufs=2))
        fin = pc.enter_context(tc.tile_pool(name="fin", bufs=1))
        qps = pc.enter_context(tc.tile_pool(name="crf_ps", bufs=2, space="PSUM"))
        fps = pc.enter_context(tc.tile_pool(name="fin_ps", bufs=1, space="PSUM"))

        emv = crf.tile([T, S * B], F32)
        nc.sync.dma_start(emv[:], em_all[:])
        oh_sb = crf.tile([T, S * B], F32)
        nc.sync.dma_start(oh_sb[:], oh[:])
        expEm = crf.tile([T, S * B], BF16)
        cbias = wp.tile([T, 1], F32)
        nc.vector.memset(cbias[:], -CLOG)
        nc.scalar.activation(expEm[:], emv[:], AF.Exp, bias=cbias[:, 0:1])

        expT_sb = wp.tile([T, T], BF16)
        nc.sync.dma_start(expT_sb[:], expT[:])
        expS_sb = wp.tile([T, 1], F32)
        nc.sync.dma_start(expS_sb[:], expS[:])
        expE_sb = wp.tile([T, 1], F32)
        nc.sync.dma_start(expE_sb[:], expE[:])
        ones_bf = wp.tile([T, 1], BF16)
        nc.vector.memset(ones_bf[:], 1.0)
        ones_f = wp.tile([T, 1], F32)
        nc.vector.memset(ones_f[:], 1.0)
        ncst = fin.tile([1, B], F32)
        nc.sync.dma_start(ncst[:], numconst[:])

        # numerator em-part: mean_b sum_t em[tag_t] == (sum over everything of
        # em*onehot) / B -- only the global mean is needed (linearity of the
        # final mean over sequences).
        s1 = crf.tile([T, S * B], F32)
        nc.vector.tensor_mul(s1[:], emv[:], oh_sb[:])
        s1red = fin.tile([T, 1], F32)
        nc.vector.reduce_sum(out=s1red[:], in_=s1[:], axis=mybir.AxisListType.X)
        invB = wp.tile([T, 1], F32)
        nc.vector.memset(invB[:], 1.0 / B)

        # denominator scan
        q_prev = qp.tile([T, B], BF16, tag="q")
        nc.vector.tensor_scalar_mul(q_prev[:], in0=expEm[:, 0:B], scalar1=expS_sb[:, 0:1])
        for t in range(1, S):
            psq = qps.tile([T, B], F32, tag="qps")
            nc.tensor.matmul(psq[:], lhsT=expT_sb[:], rhs=q_prev[:], start=True, stop=True)
            q_new = qp.tile([T, B], BF16, tag="q")
            nc.vector.tensor_mul(q_new[:], psq[:], expEm[:, t * B:(t + 1) * B])
            q_prev = q_new
        w_t = qp.tile([T, B], BF16, tag="wt")
        nc.vector.tensor_scalar_mul(w_t[:], in0=q_prev[:], scalar1=expE_sb[:, 0:1])

        psS = fps.tile([1, B], F32, tag="psS")
        nc.tensor.matmul(psS[:], lhsT=ones_bf[:], rhs=w_t[:], start=True, stop=True)
        psN = fps.tile([1, 1], F32, tag="psN")
        nc.tensor.matmul(psN[:], lhsT=invB[:], rhs=s1red[:], start=True, stop=True)

        lnS = fin.tile([1, B], F32)
        nc.scalar.activation(lnS[:], psS[:], AF.Ln)
        t2 = fin.tile([1, B], F32)
        # t2 = (lnS + S*C) - numconst   (per-sequence logZ minus host numerator part)
        sc_c = wp.tile([1, 1], F32)
        nc.vector.memset(sc_c[:], float(S) * CLOG)
        nc.vector.scalar_tensor_tensor(
            out=t2[:], in0=lnS[:], scalar=sc_c[:, 0:1],
            op0=OP.add, op1=OP.subtract, in1=ncst[:],
        )
        res = fin.tile([1, 1], F32)
        nc.vector.reduce_sum(out=res[:], in_=t2[:], axis=mybir.AxisListType.X)
        res2 = fin.tile([1, 1], F32)
        nc.scalar.mul(res2[:], res[:], 1.0 / B)
        res3 = fin.tile([1, 1], F32)
        nc.vector.tensor_sub(res3[:], res2[:], psN[:])
        nc.sync.dma_start(out[:], res3[:])


# ---------------- host-side preparation ----------------

GATE_PERM = [0, 1, 3, 2]  # torch (i,f,g,o) blocks -> our (i,f,o,g) order


def _perm_rows(w):
    b = w.reshape(4, H, *w.shape[1:])
    return b[GATE_PERM].reshape(4 * H, *w.shape[1:])


def _prep_lhsT(wm, nkt):
    """wm [4H, K] (already row-permuted) -> [P, nkt*G4] with col = kt*G4 + m*P + q."""
    K = wm.shape[1]
    assert K == nkt * P
    arr = wm.reshape(M16, P, nkt, P)          # [m, q, kt, p]
    arr = arr.transpose(3, 2, 0, 1)           # [p, kt, m, q]
    return np.ascontiguousarray(arr.reshape(P, nkt * M16 * P))


def _prep_woutT(wh):
    """wh [T, H] -> [P, KC*T] with col = kc*T + j ; element = wh[j, kc*P+p]."""
    arr = wh.T.reshape(KC, P, T)              # [kc, p, j]
    arr = arr.transpose(1, 0, 2)              # [p, kc, j]
    return np.ascontiguousarray(arr.reshape(P, KC * T))


_NC_CACHE = {}


def _get_nc(S):
    if S not in _NC_CACHE:
        _NC_CACHE[S] = build_nc(S)
    return _NC_CACHE[S]


def make_in_maps(inputs, S):
    SC = S // NCORE
    TOK = SC * B
    NG = max(1, TOK // P)

    f32 = lambda a: np.ascontiguousarray(np.asarray(a, np.float32))
    bf16 = lambda a: np.ascontiguousarray(
        np.asarray(a, np.float32).astype(ml_dtypes.bfloat16))

    x = np.asarray(inputs["x"]).astype(np.int64)[:, :S]
    tags = np.asarray(inputs["tags"]).astype(np.int64)[:, :S]
    emb = f32(inputs["emb"])
    Wih_f, Whh_f = f32(inputs["Wih_f"]), f32(inputs["Whh_f"])
    Wih_b, Whh_b = f32(inputs["Wih_b"]), f32(inputs["Whh_b"])
    bih_f, bhh_f = f32(inputs["bih_f"]), f32(inputs["bhh_f"])
    bih_b, bhh_b = f32(inputs["bih_b"]), f32(inputs["bhh_b"])
    Wout, bout = f32(inputs["Wout"]), f32(inputs["bout"])
    start, end, trans = f32(inputs["start"]), f32(inputs["end"]), f32(inputs["trans"])

    id128 = bf16(np.eye(P, dtype=np.float32))
    wihT_f = bf16(_prep_lhsT(_perm_rows(Wih_f), KT))
    wihT_b = bf16(_prep_lhsT(_perm_rows(Wih_b), KT))
    whhT_f = bf16(_prep_lhsT(_perm_rows(Whh_f), KC))
    whhT_b = bf16(_prep_lhsT(_perm_rows(Whh_b), KC))
    pb_f = f32(_perm_rows((bih_f + bhh_f)[:, None])[:, 0].reshape(M16, P).T)
    pb_b = f32(_perm_rows((bih_b + bhh_b)[:, None])[:, 0].reshape(M16, P).T)
    woutT_f = bf16(_prep_woutT(Wout[:, :H]))
    woutT_b = bf16(_prep_woutT(Wout[:, H:]))
    wout_z = np.zeros_like(woutT_f)
    bout_c = f32(bout[:, None])
    bout_z = np.zeros_like(bout_c)
    expTm = bf16(np.exp(trans))
    expSv = f32(np.exp(start)[:, None])
    expEv = f32(np.exp(end)[:, None])

    xs = x.T  # [S, B] time-major
    ohm = f32(tags.T.reshape(S * B)[None, :] == np.arange(T)[:, None])
    numconst = f32(
        start[tags[:, 0]]
        + trans[tags[:, :-1], tags[:, 1:]].sum(1)
        + end[tags[:, -1]]
    )[None, :]

    in_maps = []
    for r in range(NCORE):
        d = r % 2
        xf = xs[SC * r: SC * (r + 1)].reshape(TOK).astype(np.int32)
        s_r = (r - 1) % NCORE
        karr = np.arange(SC * s_r, SC * (s_r + 1))
        xb = xs[S - 1 - karr].reshape(TOK).astype(np.int32)

        m_idx = np.arange(M16)[:, None] * P + np.arange(P)[None, :]   # [M16, P]
        xprows0 = (d * (M16 * P) + m_idx).astype(np.int32)[:, :, None]
        rg = np.zeros((NCORE - 1, M16, P), np.int32)
        for u in range(1, NCORE):
            rank = u if d == 0 else (u + 1) % NCORE
            rg[u - 1] = ((rank * 2 + d) * M16) * P + m_idx
        xprowsg = np.ascontiguousarray(rg[:, :, :, None])

        in_maps.append(dict(
            emb=emb, id128=id128,
            xf_idx=np.ascontiguousarray(xf.reshape(NG, P, 1)),
            xb_idx=np.ascontiguousarray(xb.reshape(NG, P, 1)),
            wihT_f=wihT_f, wihT_b=wihT_b, pbias_f=pb_f, pbias_b=pb_b,
            whhT=(whhT_f if d == 0 else whhT_b),
            xprows0=xprows0, xprowsg=xprowsg,
            woutT_A=(woutT_f if r == 0 else wout_z),
            woutT_B=(woutT_b if r == 1 else wout_z),
            bout_h=(bout_c if r == 0 else bout_z),
            expT=expTm, expS=expSv, expE=expEv, oh=ohm, numconst=numconst,
        ))
    return in_maps


def kernel(**inputs):
    S = int(np.asarray(inputs["x"]).shape[1])
    nc = _get_nc(S)
    in_maps = make_in_maps(inputs, S)
    res = run_bass_kernel_spmd(nc, in_maps, list(range(NCORE)))
    val = np.float32(res.results[0]["out"][0, 0])
    return np.asarray(val, dtype=np.float32)
